# revision 9
# baseline (speedup 1.0000x reference)
"""Distributed real SHT (spherical harmonic transform) on 8 trn2 NeuronCores.

Pipeline:
  out[b,c,l,m] = sum_k W[m,l,k] * XF[b,c,m,k],   XF = (2*pi/nlon) * rfft(x, lon)[..., :mmax]

Stage A (launch 1, channel-sharded): DFT along longitude as bf16 matmuls.
  Host folds x over lon parity (cos: n'=0..360, sin: n'=1..359) and packs
  GROUPS of 4 channels per DMA so every transfer is >=0.6 MB with >=2.9 KB
  contiguous per-partition runs (descriptor-efficient; single-queue BW was
  measured 112 GB/s at 0.7 KB runs vs 200 GB/s at 4.3 KB).
  psum[k_tile, m] += xT[n'chunk, k_tile]^T @ DFTmat[n'chunk, m]
Host exchange: XF[c,k,m] (channel-sharded) -> per-core m-sharded, p-major
  chunk layout (partition-major so per-partition runs are nkc*1KB).
Stage B (launch 2, m-sharded): psum[l_tile, 512] += WT[k, l_tile]^T @ XFB[k, 512]
  Only the latitude window where P_l^m is non-negligible is loaded and
  contracted, with EXACT spans (partial last chunk via a separate DMA +
  partial-partition matmul) instead of 128-row padding.

All DMA traffic is spread across the three queues (sync/scalar HWDGE +
gpsimd SWDGE) roughly by their measured throughputs. bf16 operands keep
the PE at 2.4 GHz and halve DMA bytes; psum accumulation is fp32.
"""

import os

import numpy as np

import concourse.bacc as bacc
import concourse.mybir as mybir
from concourse.tile import TileContext
from concourse.bass_utils import run_bass_kernel_spmd

LAST_PERF = {}

NLAT = 361
NLON = 720
MMAX = 361
LMAX = 361
C = 256
NCORES = 8
CPC = C // NCORES  # 32 channels per core
NC_COS = NLON // 2 + 1  # 361 cos columns (n'=0..360)
NC_SIN = NLON // 2 - 1  # 359 sin columns (n'=1..359)
MPC = (MMAX + NCORES - 1) // NCORES  # 46 m's per core (padded)
MEVEN = 362  # m padded even (stage A moving free dim)
GA = 4  # channels per stage-A DMA group
NGA = CPC // GA  # 8 groups per core
NKC_MAX = 3  # max 128-row latitude chunks in stage B
NRIC = 2 * C  # 512 = (re|im) x 256 channels

F32 = mybir.dt.float32
BF16 = mybir.dt.bfloat16

K_TILES = [(0, 128), (128, 128), (256, 105)]


def _ptiles(n, p=128):
    out = []
    o = 0
    while o < n:
        out.append((o, min(p, n - o)))
        o += p
    return out


def build_stage_a():
    """xin [NGA, 768, GA*362] bf16: row r = DFT contraction row (cos rows
    0:361 in segs 0-2, sin rows 384:743 in segs 3-5), col = c*362 + k.
    mats [768, 362] bf16 (same row packing, cols = m, col 361 zero).
    xf [NGA, 361, GA*724]: row k, col = c*724 + ri*362 + m."""
    nc = bacc.Bacc("TRN2", target_bir_lowering=False)
    xin = nc.dram_tensor("xin", [NGA, 768, GA * MEVEN], BF16, kind="ExternalInput")
    mats = nc.dram_tensor("mats", [768, MEVEN], BF16, kind="ExternalInput")
    xf = nc.dram_tensor("xf", [NGA, NLAT, GA * 2 * MEVEN], BF16, kind="ExternalOutput")

    with TileContext(nc) as tc:
        with (
            tc.tile_pool(name="mats", bufs=1) as matp,
            tc.tile_pool(name="xinp", bufs=3) as xinp,
            tc.tile_pool(name="outp", bufs=6) as outp,
            tc.tile_pool(name="ps", bufs=6, space="PSUM") as psp,
        ):
            mat_t = matp.tile([128, 6 * MEVEN], BF16, tag="mats")
            nc.sync.dma_start(
                out=mat_t.rearrange("p (s m) -> p s m", s=6),
                in_=mats.rearrange("(s p) m -> p s m", p=128),
            )
            copy_i = 0
            store_i = 0
            # stores: ~1/2 gpsimd, 1/4 sync, 1/4 scalar (throughput-weighted)
            store_rot = (nc.gpsimd, nc.sync, nc.gpsimd, nc.scalar)
            for g in range(NGA):
                x_t = xinp.tile([128, 6 * GA * MEVEN], BF16, tag="xin")
                (nc.sync if g % 2 == 0 else nc.scalar).dma_start(
                    out=x_t.rearrange("p (s f) -> p s f", s=6),
                    in_=xin[g].rearrange("(s p) f -> p s f", p=128),
                )
                ots = [
                    outp.tile([128, GA * 2 * MEVEN], BF16, tag="ot", name=f"ot{kt}")
                    for kt in range(len(K_TILES))
                ]
                for c in range(GA):
                    for ri in range(2):
                        for kt, (k0, kp) in enumerate(K_TILES):
                            ps = psp.tile([128, MEVEN], F32, tag="ps")
                            for s in range(3):
                                seg = 3 * ri + s
                                base = (seg * GA + c) * MEVEN
                                nc.tensor.matmul(
                                    ps[:kp, :],
                                    x_t[:, base + k0 : base + k0 + kp],
                                    mat_t[:, seg * MEVEN : (seg + 1) * MEVEN],
                                    start=(s == 0),
                                    stop=(s == 2),
                                )
                            dst = ots[kt][:kp, (c * 2 + ri) * MEVEN : (c * 2 + ri + 1) * MEVEN]
                            if copy_i % 3 != 2:  # 2/3 DVE, 1/3 ACT (ACT is ~2x slower)
                                nc.vector.tensor_copy(out=dst, in_=ps[:kp, :])
                            else:
                                nc.scalar.copy(dst, ps[:kp, :])
                            copy_i += 1
                for kt, (k0, kp) in enumerate(K_TILES):
                    store_rot[store_i % 4].dma_start(
                        out=xf[g, k0 : k0 + kp, :], in_=ots[kt][:kp, :]
                    )
                    store_i += 1
    nc.compile()
    return nc


def build_stage_b(nkc_list, r_list):
    """Index i handles m = 8*i + core_j; computes l in [8*i, lmax).
    xfb [MPC, 128, 3*512] bf16, p-major: col t*512+f holds XF latitude row
    (klo_i + t*128 + p), f = ri*256 + c.  wt [MPC, 128, 1088]: col t*Lp+lc
    holds W[m, same k row, l_lo_i + lc] (Lp = lmax - 8*(i & ~1); odd i's
    last 8 cols zero).  outb [MPC, 128, 3*512]: col t*512+f holds out row
    l = 8*i + t*128 + p.
    nkc_list[i]: latitude chunks (uniform within each index pair);
    r_list[i]: valid rows of the last chunk (span - 128*(nkc-1))."""
    nc = bacc.Bacc("TRN2", target_bir_lowering=False)
    xfb = nc.dram_tensor("xfb", [MPC, 128, NKC_MAX * NRIC], BF16, kind="ExternalInput")
    wt = nc.dram_tensor("wt", [MPC, 128, 1088], BF16, kind="ExternalInput")
    outb = nc.dram_tensor("outb", [MPC, 128, NKC_MAX * NRIC], BF16, kind="ExternalOutput")

    with TileContext(nc) as tc:
        with (
            tc.tile_pool(name="rhs", bufs=3) as rhsp,
            tc.tile_pool(name="wts", bufs=3) as wtp,
            tc.tile_pool(name="outp", bufs=3) as outp,
            tc.tile_pool(name="ps", bufs=7, space="PSUM") as psp,
        ):
            copy_i = 0
            for pi in range(0, MPC, 2):
                nkc = nkc_list[pi]
                Lp = LMAX - 8 * pi  # shared col width for the pair
                x_t = rhsp.tile([128, 2 * NKC_MAX * NRIC], BF16, tag="rhs")
                w_t = wtp.tile([128, 2 * 1088], BF16, tag="wt")
                ot = outp.tile([128, 2 * NKC_MAX * NRIC], BF16, tag="ot")
                eng_a = nc.sync if (pi // 2) % 2 == 0 else nc.scalar
                eng_b = nc.scalar if (pi // 2) % 2 == 0 else nc.sync
                if nkc > 1:  # full chunks, both indices in one transfer
                    eng_a.dma_start(
                        out=x_t.rearrange("p (i f) -> p i f", i=2)[:, :, : (nkc - 1) * NRIC],
                        in_=xfb[pi : pi + 2, :, : (nkc - 1) * NRIC].rearrange(
                            "i p f -> p i f"
                        ),
                    )
                    eng_b.dma_start(
                        out=w_t.rearrange("p (i f) -> p i f", i=2)[:, :, : (nkc - 1) * Lp],
                        in_=wt[pi : pi + 2, :, : (nkc - 1) * Lp].rearrange("i p f -> p i f"),
                    )
                for il in range(2):
                    r = r_list[pi + il]
                    (eng_a if il == 0 else eng_b).dma_start(
                        out=x_t[
                            :r,
                            il * NKC_MAX * NRIC
                            + (nkc - 1) * NRIC : il * NKC_MAX * NRIC
                            + nkc * NRIC,
                        ],
                        in_=xfb[pi + il, :r, (nkc - 1) * NRIC : nkc * NRIC],
                    )
                    (eng_b if il == 0 else eng_a).dma_start(
                        out=w_t[:r, il * 1088 + (nkc - 1) * Lp : il * 1088 + nkc * Lp],
                        in_=wt[pi + il, :r, (nkc - 1) * Lp : nkc * Lp],
                    )
                for il in range(2):
                    i = pi + il
                    Li = LMAX - 8 * i
                    for tl, (l0, lp) in enumerate(_ptiles(Li)):
                        ps = psp.tile([128, NRIC], F32, tag="ps")
                        for t in range(nkc):
                            rt = 128 if t < nkc - 1 else r_list[i]
                            nc.tensor.matmul(
                                ps[:lp, :],
                                w_t[:rt, il * 1088 + t * Lp + l0 : il * 1088 + t * Lp + l0 + lp],
                                x_t[:rt, il * NKC_MAX * NRIC + t * NRIC : il * NKC_MAX * NRIC + (t + 1) * NRIC],
                                start=(t == 0),
                                stop=(t == nkc - 1),
                            )
                        dst = ot[:lp, (il * NKC_MAX + tl) * NRIC : (il * NKC_MAX + tl + 1) * NRIC]
                        if copy_i % 3 != 2:  # 2/3 DVE, 1/3 ACT
                            nc.vector.tensor_copy(out=dst, in_=ps[:lp, :])
                        else:
                            nc.scalar.copy(dst, ps[:lp, :])
                        copy_i += 1
                # stores: full l-tiles batched per index (big -> gpsimd),
                # partial tile separate (small -> HWDGE)
                for il in range(2):
                    i = pi + il
                    tiles = _ptiles(LMAX - 8 * i)
                    nfull = len(tiles) - 1
                    if nfull > 0:
                        nc.gpsimd.dma_start(
                            out=outb[i, :, : nfull * NRIC],
                            in_=ot[:, il * NKC_MAX * NRIC : il * NKC_MAX * NRIC + nfull * NRIC],
                        )
                    lp_last = tiles[-1][1]
                    (eng_a if il == 0 else eng_b).dma_start(
                        out=outb[i, :lp_last, nfull * NRIC : (nfull + 1) * NRIC],
                        in_=ot[
                            :lp_last,
                            (il * NKC_MAX + nfull) * NRIC : (il * NKC_MAX + nfull + 1) * NRIC,
                        ],
                    )
    nc.compile()
    return nc


def _dft_matrices():
    """cosm[n', m] = s*cos(2 pi m n'/nlon), n'=0..360
    sinm[n', m] = -s*sin(2 pi m n'/nlon), n'=1..359 (imag of rfft = -sum x sin)."""
    s = 2.0 * np.pi / NLON
    m = np.arange(MMAX)
    nc_ = np.arange(NC_COS)
    ns_ = np.arange(1, NLON // 2)
    ang_c = 2.0 * np.pi * ((nc_[:, None] * m[None, :]) % NLON) / NLON
    ang_s = 2.0 * np.pi * ((ns_[:, None] * m[None, :]) % NLON) / NLON
    return (s * np.cos(ang_c)).astype(np.float32), (-s * np.sin(ang_s)).astype(
        np.float32
    )


def fold_x(x):
    """x: (C, nlat, nlon) f32 -> xc (C, nlat, 361), xs (C, nlat, 359)."""
    xc = np.empty((x.shape[0], x.shape[1], NC_COS), dtype=np.float32)
    xc[..., 0] = x[..., 0]
    xc[..., NLON // 2] = x[..., NLON // 2]
    xc[..., 1 : NLON // 2] = x[..., 1 : NLON // 2] + x[..., : NLON // 2 : -1]
    xs = x[..., 1 : NLON // 2] - x[..., : NLON // 2 : -1]
    return xc, np.ascontiguousarray(xs.astype(np.float32))


def pack_stage_a_inputs(x):
    """x: (C, nlat, nlon) f32 -> xin (C//GA, 768, GA*362) bf16, mats (768, 362)."""
    import ml_dtypes

    bf = ml_dtypes.bfloat16
    xc, xs = fold_x(x)  # (C, k, n')
    ng = x.shape[0] // GA
    xin = np.zeros((ng, 768, GA, MEVEN), dtype=bf)
    # [g, n', c, k] <- transpose of (g, c, k, n')
    xin[:, :NC_COS, :, :NLAT] = (
        xc.reshape(ng, GA, NLAT, NC_COS).transpose(0, 3, 1, 2).astype(bf)
    )
    xin[:, 384 : 384 + NC_SIN, :, :NLAT] = (
        xs.reshape(ng, GA, NLAT, NC_SIN).transpose(0, 3, 1, 2).astype(bf)
    )
    cosm, sinm = _dft_matrices()
    mats = np.zeros((768, MEVEN), dtype=bf)
    mats[:NC_COS, :MMAX] = cosm.astype(bf)
    mats[384 : 384 + NC_SIN, :MMAX] = sinm.astype(bf)
    return xin.reshape(ng, 768, GA * MEVEN), mats


def _windows(weights):
    """Per index-pair latitude windows: union of |W| support over the 8
    cores' m's, span forced uniform (in chunk count) within each pair."""
    wabs = np.abs(weights).max(axis=1)  # (m, k)
    thr = 1e-7 * wabs.max()
    win = []
    for i in range(MPC):
        ms = [NCORES * i + j for j in range(NCORES) if NCORES * i + j < MMAX]
        nz = np.nonzero(wabs[ms].max(axis=0) > thr)[0]
        klo, khi = (int(nz[0]), int(nz[-1]) + 1) if len(nz) else (0, NLAT)
        win.append([klo, khi])
    nkc_list, r_list, klo_list = [], [], []
    for pi in range(0, MPC, 2):
        nkc = max(-(-(w[1] - w[0]) // 128) for w in win[pi : pi + 2])
        for i in (pi, pi + 1):
            klo, khi = win[i]
            need = (nkc - 1) * 128 + 16  # keep the partial chunk >= 16 rows
            if khi - klo < need:
                khi = min(NLAT, klo + need)
                klo = max(0, khi - need)
            nkc_list.append(nkc)
            r_list.append(khi - klo - 128 * (nkc - 1))
            klo_list.append(klo)
    return nkc_list, r_list, klo_list


def m_list(j):
    return [NCORES * i + j for i in range(MPC) if NCORES * i + j < MMAX]


def _install_ntff_hook():
    """This image's antenv lacks axon_hooks; synthesize it so bass_utils'
    trace=True path can capture NTFFs via the axon PJRT .so."""
    import sys

    if "antenv.axon_hooks" in sys.modules:
        return
    import types

    mod = types.ModuleType("antenv.axon_hooks")
    state = {"hook": None}
    mod.set_axon_ntff_profile_hook = lambda h: state.__setitem__("hook", h)
    mod.get_axon_ntff_profile_hook = lambda: state["hook"]
    sys.modules["antenv.axon_hooks"] = mod
    try:
        import importlib.util as ilu

        spec = ilu.spec_from_file_location(
            "_trn_boot_hook", "/root/.axon_site/trn_agent_boot/trn_boot.py"
        )
        tb = ilu.module_from_spec(spec)
        spec.loader.exec_module(tb)
        mod.set_axon_ntff_profile_hook(
            tb._ntff_profile_via_ctypes("/opt/axon/libaxon_pjrt.so")
        )
    except Exception:
        pass


def _run(nc, in_maps, label):
    kw = {}
    if os.environ.get("SHT_TRACE"):
        import concourse.bass_utils as bu

        bu.upload_artifacts = lambda tmpdir: tmpdir  # no S3 in this sandbox
        _install_ntff_hook()
        kw = dict(trace=True)
    try:
        res = run_bass_kernel_spmd(nc, in_maps, core_ids=list(range(NCORES)), **kw)
    except Exception:
        if not kw:
            raise
        res = run_bass_kernel_spmd(nc, in_maps, core_ids=list(range(NCORES)))
    LAST_PERF[label] = res.exec_time_ns
    return res


def kernel(x, weights):
    import ml_dtypes

    bf = ml_dtypes.bfloat16
    x = np.asarray(x, dtype=np.float32).reshape(C, NLAT, NLON)
    weights = np.asarray(weights, dtype=np.float32)

    xin, mats = pack_stage_a_inputs(x)
    nc_a = build_stage_a()
    in_maps = [
        {"xin": xin[j * NGA : (j + 1) * NGA], "mats": mats} for j in range(NCORES)
    ]
    res_a = _run(nc_a, in_maps, "stage_a")
    # reassemble XF[c, k, m] re/im from [g, k, c*724 + ri*362 + m]
    xf_all = np.concatenate(
        [np.asarray(r["xf"]).reshape(NGA, NLAT, GA, 2, MEVEN) for r in res_a.results],
        axis=0,
    )  # (C//GA, k, GA, ri, m)
    xf_all = xf_all.transpose(0, 2, 3, 1, 4).reshape(C, 2, NLAT, MEVEN)
    xfr = np.ascontiguousarray(xf_all[:, 0, :, :MMAX])  # (C, k, m) bf16
    xfi = np.ascontiguousarray(xf_all[:, 1, :, :MMAX])

    nkc_list, r_list, klo_list = _windows(weights)
    wtf = weights.transpose(0, 2, 1)  # (m, k, l) f32
    in_maps_b = []
    for j in range(NCORES):
        xfb = np.zeros((MPC, 128, NKC_MAX * NRIC), dtype=bf)
        wtj = np.zeros((MPC, 128, 1088), dtype=bf)
        for i in range(MPC):
            m = NCORES * i + j
            if m >= MMAX:
                continue
            nkc, klo = nkc_list[i], klo_list[i]
            span = 128 * (nkc - 1) + r_list[i]
            khi = klo + span
            Lp = LMAX - 8 * (i & ~1)
            l_lo = 8 * i
            # (span, 512) -> chunk-padded (nkc*128, 512) -> p-major (128, nkc, 512)
            src = np.empty((span, NRIC), dtype=bf)
            src[:, :C] = xfr[:, klo:khi, m].T
            src[:, C:] = xfi[:, klo:khi, m].T
            pad = np.zeros((nkc * 128, NRIC), dtype=bf)
            pad[:span] = src
            xfb[i, :, : nkc * NRIC] = pad.reshape(nkc, 128, NRIC).transpose(1, 0, 2).reshape(128, nkc * NRIC)
            wsrc = np.zeros((nkc * 128, Lp), dtype=bf)
            wsrc[:span, : LMAX - l_lo] = wtf[m, klo:khi, l_lo:].astype(bf)
            wtj[i, :, : nkc * Lp] = wsrc.reshape(nkc, 128, Lp).transpose(1, 0, 2).reshape(128, nkc * Lp)
        in_maps_b.append({"xfb": xfb, "wt": wtj})
    nc_b = build_stage_b(nkc_list, r_list)
    res_b = _run(nc_b, in_maps_b, "stage_b")

    out = np.zeros((1, C, LMAX, MMAX), dtype=np.complex64)
    for j in range(NCORES):
        o = np.asarray(res_b.results[j]["outb"], dtype=np.float32)  # (MPC,128,1536)
        o = o.reshape(MPC, 128, NKC_MAX, NRIC).transpose(0, 2, 1, 3)  # (i, tl, p, f)
        for i in range(MPC):
            m = NCORES * i + j
            if m >= MMAX:
                continue
            Li = LMAX - 8 * i
            flat = o[i].reshape(NKC_MAX * 128, NRIC)[:Li]  # (l - 8i, f)
            out[0, :, 8 * i :, m] = (flat[:, :C] + 1j * flat[:, C:]).T
    return out


# revision 13
# speedup vs baseline: 1.4531x; 1.4531x over previous
"""Distributed real SHT (spherical harmonic transform) on 8 trn2 NeuronCores.

Pipeline:
  out[b,c,l,m] = sum_k W[m,l,k] * XF[b,c,m,k],   XF = (2*pi/nlon) * rfft(x, lon)[..., :mmax]

Stage A (launch 1, channel-sharded): DFT along longitude as bf16 matmuls.
  Host folds x over lon parity (cos: n'=0..360, sin: n'=1..359) and packs
  GROUPS of 4 channels per DMA so every transfer is >=0.6 MB with >=2.9 KB
  contiguous per-partition runs (descriptor-efficient; single-queue BW was
  measured 112 GB/s at 0.7 KB runs vs 200 GB/s at 4.3 KB).
  psum[k_tile, m] += xT[n'chunk, k_tile]^T @ DFTmat[n'chunk, m]
Host exchange: XF[c,k,m] (channel-sharded) -> per-core m-sharded, p-major
  chunk layout (partition-major so per-partition runs are nkc*1KB).
Stage B (launch 2, m-sharded): psum[l_tile, 512] += WT[k, l_tile]^T @ XFB[k, 512]
  Only the latitude window where P_l^m is non-negligible is loaded and
  contracted, with EXACT spans (partial last chunk via a separate DMA +
  partial-partition matmul) instead of 128-row padding.

All DMA traffic is spread across the three queues (sync/scalar HWDGE +
gpsimd SWDGE) roughly by their measured throughputs. bf16 operands keep
the PE at 2.4 GHz and halve DMA bytes; psum accumulation is fp32.
"""

import os

import numpy as np

import concourse.bacc as bacc
import concourse.mybir as mybir
from concourse.tile import TileContext
from concourse.bass_utils import run_bass_kernel_spmd

LAST_PERF = {}

NLAT = 361
NLON = 720
MMAX = 361
LMAX = 361
C = 256
NCORES = 8
CPC = C // NCORES  # 32 channels per core
NC_COS = NLON // 2 + 1  # 361 cos columns (n'=0..360)
NC_SIN = NLON // 2 - 1  # 359 sin columns (n'=1..359)
MPC = (MMAX + NCORES - 1) // NCORES  # 46 m's per core (padded)
MEVEN = 362  # m padded even (stage A moving free dim)
GA = 4  # channels per stage-A DMA group
NGA = CPC // GA  # 8 groups per core
NKC_MAX = 3  # max 128-row latitude chunks in stage B
NRIC = 2 * C  # 512 = (re|im) x 256 channels

F32 = mybir.dt.float32
BF16 = mybir.dt.bfloat16

K_TILES = [(0, 128), (128, 128), (256, 105)]


def _ptiles(n, p=128):
    out = []
    o = 0
    while o < n:
        out.append((o, min(p, n - o)))
        o += p
    return out


def build_stage_a():
    """xin [NGA, 768, GA*362] bf16: row r = DFT contraction row (cos rows
    0:361 in segs 0-2, sin rows 384:743 in segs 3-5), col = c*362 + k.
    mats [768, 362] bf16 (same row packing, cols = m, col 361 zero).
    xf [NGA, 361, GA*724]: row k, col = c*724 + ri*362 + m."""
    nc = bacc.Bacc("TRN2", target_bir_lowering=False)
    xin = nc.dram_tensor("xin", [NGA, 768, GA * MEVEN], BF16, kind="ExternalInput")
    mats = nc.dram_tensor("mats", [768, MEVEN], BF16, kind="ExternalInput")
    xf = nc.dram_tensor("xf", [NGA, NLAT, GA * 2 * MEVEN], BF16, kind="ExternalOutput")

    with TileContext(nc) as tc:
        with (
            tc.tile_pool(name="mats", bufs=1) as matp,
            tc.tile_pool(name="xinp", bufs=4) as xinp,
            tc.tile_pool(name="outp", bufs=6) as outp,
            tc.tile_pool(name="ps", bufs=6, space="PSUM") as psp,
        ):
            mat_t = matp.tile([128, 6 * MEVEN], BF16, tag="mats")
            nc.sync.dma_start(
                out=mat_t.rearrange("p (s m) -> p s m", s=6),
                in_=mats.rearrange("(s p) m -> p s m", p=128),
            )
            copy_i = 0
            for g in range(NGA):
                x_t = xinp.tile([128, 6 * GA * MEVEN], BF16, tag="xin")
                (nc.sync if g % 2 == 0 else nc.scalar).dma_start(
                    out=x_t.rearrange("p (s f) -> p s f", s=6),
                    in_=xin[g].rearrange("(s p) f -> p s f", p=128),
                )
                ots = [
                    outp.tile([128, GA * 2 * MEVEN], BF16, tag="ot", name=f"ot{kt}")
                    for kt in range(len(K_TILES))
                ]
                for c in range(GA):
                    for ri in range(2):
                        for kt, (k0, kp) in enumerate(K_TILES):
                            ps = psp.tile([128, MEVEN], F32, tag="ps")
                            for s in range(3):
                                seg = 3 * ri + s
                                base = (seg * GA + c) * MEVEN
                                nc.tensor.matmul(
                                    ps[:kp, :],
                                    x_t[:, base + k0 : base + k0 + kp],
                                    mat_t[:, seg * MEVEN : (seg + 1) * MEVEN],
                                    start=(s == 0),
                                    stop=(s == 2),
                                )
                            dst = ots[kt][:kp, (c * 2 + ri) * MEVEN : (c * 2 + ri + 1) * MEVEN]
                            if copy_i % 3 != 2:  # 2/3 DVE, 1/3 ACT (ACT is ~2x slower)
                                nc.vector.tensor_copy(out=dst, in_=ps[:kp, :])
                            else:
                                nc.scalar.copy(dst, ps[:kp, :])
                            copy_i += 1
                for kt, (k0, kp) in enumerate(K_TILES):
                    # kt0/kt1 -> gpsimd; kt2 -> the HWDGE engine that is NOT
                    # about to issue the next group's load
                    if kt < 2:
                        st = nc.gpsimd
                    else:
                        st = nc.scalar if g % 2 == 0 else nc.sync
                    st.dma_start(out=xf[g, k0 : k0 + kp, :], in_=ots[kt][:kp, :])
    nc.compile()
    return nc


def build_stage_b(nkc_list, r_list):
    """Index i handles m = 8*i + core_j; computes l in [8*i, lmax).
    xfb [MPC, 128, 3*512] bf16, p-major: col t*512+f holds XF latitude row
    (klo_i + t*128 + p), f = ri*256 + c.  wt [MPC, 128, 1088]: col t*Lp+lc
    holds W[m, same k row, l_lo_i + lc] (Lp = lmax - 8*(i & ~1); odd i's
    last 8 cols zero).  outb [MPC, 128, 3*512]: col t*512+f holds out row
    l = 8*i + t*128 + p.
    nkc_list[i]: latitude chunks (uniform within each index pair);
    r_list[i]: valid rows of the last chunk (span - 128*(nkc-1))."""
    nc = bacc.Bacc("TRN2", target_bir_lowering=False)
    xfb = nc.dram_tensor("xfb", [MPC, 128, NKC_MAX * NRIC], BF16, kind="ExternalInput")
    wt = nc.dram_tensor("wt", [MPC, 128, 1088], BF16, kind="ExternalInput")
    outb = nc.dram_tensor("outb", [MPC, 128, NKC_MAX * NRIC], BF16, kind="ExternalOutput")

    with TileContext(nc) as tc:
        with (
            tc.tile_pool(name="rhs", bufs=6) as rhsp,
            tc.tile_pool(name="wts", bufs=6) as wtp,
            tc.tile_pool(name="outp", bufs=6) as outp,
            tc.tile_pool(name="ps", bufs=7, space="PSUM") as psp,
        ):
            copy_i = 0
            for i in range(MPC):
                nkc = nkc_list[i]
                Lp = LMAX - 8 * (i & ~1)  # col width shared within the pair
                Li = LMAX - 8 * i
                x_t = rhsp.tile([128, NKC_MAX * NRIC], BF16, tag="rhs")
                w_t = wtp.tile([128, 1088], BF16, tag="wt")
                ot = outp.tile([128, NKC_MAX * NRIC], BF16, tag="ot")
                eng_a = nc.sync if i % 2 == 0 else nc.scalar
                eng_b = nc.scalar if i % 2 == 0 else nc.sync
                # whole-index loads; last-chunk rows beyond the window are
                # host-zero-filled in BOTH tensors, so full-128 contraction
                # is exact. 2-3 KB contiguous per partition per transfer.
                eng_a.dma_start(out=x_t[:, : nkc * NRIC], in_=xfb[i, :, : nkc * NRIC])
                eng_b.dma_start(out=w_t[:, : nkc * Lp], in_=wt[i, :, : nkc * Lp])
                for tl, (l0, lp) in enumerate(_ptiles(Li)):
                    ps = psp.tile([128, NRIC], F32, tag="ps")
                    for t in range(nkc):
                        nc.tensor.matmul(
                            ps[:lp, :],
                            w_t[:, t * Lp + l0 : t * Lp + l0 + lp],
                            x_t[:, t * NRIC : (t + 1) * NRIC],
                            start=(t == 0),
                            stop=(t == nkc - 1),
                        )
                    dst = ot[:lp, tl * NRIC : (tl + 1) * NRIC]
                    if copy_i % 3 != 2:  # 2/3 DVE, 1/3 ACT (ACT is ~2x slower)
                        nc.vector.tensor_copy(out=dst, in_=ps[:lp, :])
                    else:
                        nc.scalar.copy(dst, ps[:lp, :])
                    copy_i += 1
                # stores on gpsimd only: never block the HWDGE load queues
                tiles = _ptiles(Li)
                nfull = len(tiles) - 1
                if nfull > 0:
                    nc.gpsimd.dma_start(
                        out=outb[i, :, : nfull * NRIC], in_=ot[:, : nfull * NRIC]
                    )
                lp_last = tiles[-1][1]
                nc.gpsimd.dma_start(
                    out=outb[i, :lp_last, nfull * NRIC : (nfull + 1) * NRIC],
                    in_=ot[:lp_last, nfull * NRIC : (nfull + 1) * NRIC],
                )
    nc.compile()
    return nc


def _dft_matrices():
    """cosm[n', m] = s*cos(2 pi m n'/nlon), n'=0..360
    sinm[n', m] = -s*sin(2 pi m n'/nlon), n'=1..359 (imag of rfft = -sum x sin)."""
    s = 2.0 * np.pi / NLON
    m = np.arange(MMAX)
    nc_ = np.arange(NC_COS)
    ns_ = np.arange(1, NLON // 2)
    ang_c = 2.0 * np.pi * ((nc_[:, None] * m[None, :]) % NLON) / NLON
    ang_s = 2.0 * np.pi * ((ns_[:, None] * m[None, :]) % NLON) / NLON
    return (s * np.cos(ang_c)).astype(np.float32), (-s * np.sin(ang_s)).astype(
        np.float32
    )


def fold_x(x):
    """x: (C, nlat, nlon) f32 -> xc (C, nlat, 361), xs (C, nlat, 359)."""
    xc = np.empty((x.shape[0], x.shape[1], NC_COS), dtype=np.float32)
    xc[..., 0] = x[..., 0]
    xc[..., NLON // 2] = x[..., NLON // 2]
    xc[..., 1 : NLON // 2] = x[..., 1 : NLON // 2] + x[..., : NLON // 2 : -1]
    xs = x[..., 1 : NLON // 2] - x[..., : NLON // 2 : -1]
    return xc, np.ascontiguousarray(xs.astype(np.float32))


def pack_stage_a_inputs(x):
    """x: (C, nlat, nlon) f32 -> xin (C//GA, 768, GA*362) bf16, mats (768, 362)."""
    import ml_dtypes

    bf = ml_dtypes.bfloat16
    xc, xs = fold_x(x)  # (C, k, n')
    ng = x.shape[0] // GA
    xin = np.zeros((ng, 768, GA, MEVEN), dtype=bf)
    # [g, n', c, k] <- transpose of (g, c, k, n')
    xin[:, :NC_COS, :, :NLAT] = (
        xc.reshape(ng, GA, NLAT, NC_COS).transpose(0, 3, 1, 2).astype(bf)
    )
    xin[:, 384 : 384 + NC_SIN, :, :NLAT] = (
        xs.reshape(ng, GA, NLAT, NC_SIN).transpose(0, 3, 1, 2).astype(bf)
    )
    cosm, sinm = _dft_matrices()
    mats = np.zeros((768, MEVEN), dtype=bf)
    mats[:NC_COS, :MMAX] = cosm.astype(bf)
    mats[384 : 384 + NC_SIN, :MMAX] = sinm.astype(bf)
    return xin.reshape(ng, 768, GA * MEVEN), mats


def _windows(weights):
    """Per index-pair latitude windows: union of |W| support over the 8
    cores' m's, span forced uniform (in chunk count) within each pair."""
    wabs = np.abs(weights).max(axis=1)  # (m, k)
    thr = 1e-7 * wabs.max()
    win = []
    for i in range(MPC):
        ms = [NCORES * i + j for j in range(NCORES) if NCORES * i + j < MMAX]
        nz = np.nonzero(wabs[ms].max(axis=0) > thr)[0]
        klo, khi = (int(nz[0]), int(nz[-1]) + 1) if len(nz) else (0, NLAT)
        win.append([klo, khi])
    nkc_list, r_list, klo_list = [], [], []
    for pi in range(0, MPC, 2):
        nkc = max(-(-(w[1] - w[0]) // 128) for w in win[pi : pi + 2])
        for i in (pi, pi + 1):
            klo, khi = win[i]
            need = (nkc - 1) * 128 + 16  # keep the partial chunk >= 16 rows
            if khi - klo < need:
                khi = min(NLAT, klo + need)
                klo = max(0, khi - need)
            nkc_list.append(nkc)
            r_list.append(khi - klo - 128 * (nkc - 1))
            klo_list.append(klo)
    return nkc_list, r_list, klo_list


def m_list(j):
    return [NCORES * i + j for i in range(MPC) if NCORES * i + j < MMAX]


def _install_ntff_hook():
    """This image's antenv lacks axon_hooks; synthesize it so bass_utils'
    trace=True path can capture NTFFs via the axon PJRT .so."""
    import sys

    if "antenv.axon_hooks" in sys.modules:
        return
    import types

    mod = types.ModuleType("antenv.axon_hooks")
    state = {"hook": None}
    mod.set_axon_ntff_profile_hook = lambda h: state.__setitem__("hook", h)
    mod.get_axon_ntff_profile_hook = lambda: state["hook"]
    sys.modules["antenv.axon_hooks"] = mod
    try:
        import importlib.util as ilu

        spec = ilu.spec_from_file_location(
            "_trn_boot_hook", "/root/.axon_site/trn_agent_boot/trn_boot.py"
        )
        tb = ilu.module_from_spec(spec)
        spec.loader.exec_module(tb)
        mod.set_axon_ntff_profile_hook(
            tb._ntff_profile_via_ctypes("/opt/axon/libaxon_pjrt.so")
        )
    except Exception:
        pass


def _run(nc, in_maps, label):
    kw = {}
    if os.environ.get("SHT_TRACE"):
        import concourse.bass_utils as bu

        bu.upload_artifacts = lambda tmpdir: tmpdir  # no S3 in this sandbox
        _install_ntff_hook()
        kw = dict(trace=True)
    try:
        res = run_bass_kernel_spmd(nc, in_maps, core_ids=list(range(NCORES)), **kw)
    except Exception:
        if not kw:
            raise
        res = run_bass_kernel_spmd(nc, in_maps, core_ids=list(range(NCORES)))
    LAST_PERF[label] = res.exec_time_ns
    return res


def kernel(x, weights):
    import ml_dtypes

    bf = ml_dtypes.bfloat16
    x = np.asarray(x, dtype=np.float32).reshape(C, NLAT, NLON)
    weights = np.asarray(weights, dtype=np.float32)

    xin, mats = pack_stage_a_inputs(x)
    nc_a = build_stage_a()
    in_maps = [
        {"xin": xin[j * NGA : (j + 1) * NGA], "mats": mats} for j in range(NCORES)
    ]
    res_a = _run(nc_a, in_maps, "stage_a")
    # reassemble XF[c, k, m] re/im from [g, k, c*724 + ri*362 + m]
    xf_all = np.concatenate(
        [np.asarray(r["xf"]).reshape(NGA, NLAT, GA, 2, MEVEN) for r in res_a.results],
        axis=0,
    )  # (C//GA, k, GA, ri, m)
    xf_all = xf_all.transpose(0, 2, 3, 1, 4).reshape(C, 2, NLAT, MEVEN)
    xfr = np.ascontiguousarray(xf_all[:, 0, :, :MMAX])  # (C, k, m) bf16
    xfi = np.ascontiguousarray(xf_all[:, 1, :, :MMAX])

    nkc_list, r_list, klo_list = _windows(weights)
    wtf = weights.transpose(0, 2, 1)  # (m, k, l) f32
    in_maps_b = []
    for j in range(NCORES):
        xfb = np.zeros((MPC, 128, NKC_MAX * NRIC), dtype=bf)
        wtj = np.zeros((MPC, 128, 1088), dtype=bf)
        for i in range(MPC):
            m = NCORES * i + j
            if m >= MMAX:
                continue
            nkc, klo = nkc_list[i], klo_list[i]
            span = 128 * (nkc - 1) + r_list[i]
            khi = klo + span
            Lp = LMAX - 8 * (i & ~1)
            l_lo = 8 * i
            # (span, 512) -> chunk-padded (nkc*128, 512) -> p-major (128, nkc, 512)
            src = np.empty((span, NRIC), dtype=bf)
            src[:, :C] = xfr[:, klo:khi, m].T
            src[:, C:] = xfi[:, klo:khi, m].T
            pad = np.zeros((nkc * 128, NRIC), dtype=bf)
            pad[:span] = src
            xfb[i, :, : nkc * NRIC] = pad.reshape(nkc, 128, NRIC).transpose(1, 0, 2).reshape(128, nkc * NRIC)
            wsrc = np.zeros((nkc * 128, Lp), dtype=bf)
            wsrc[:span, : LMAX - l_lo] = wtf[m, klo:khi, l_lo:].astype(bf)
            wtj[i, :, : nkc * Lp] = wsrc.reshape(nkc, 128, Lp).transpose(1, 0, 2).reshape(128, nkc * Lp)
        in_maps_b.append({"xfb": xfb, "wt": wtj})
    nc_b = build_stage_b(nkc_list, r_list)
    res_b = _run(nc_b, in_maps_b, "stage_b")

    out = np.zeros((1, C, LMAX, MMAX), dtype=np.complex64)
    for j in range(NCORES):
        o = np.asarray(res_b.results[j]["outb"], dtype=np.float32)  # (MPC,128,1536)
        o = o.reshape(MPC, 128, NKC_MAX, NRIC).transpose(0, 2, 1, 3)  # (i, tl, p, f)
        for i in range(MPC):
            m = NCORES * i + j
            if m >= MMAX:
                continue
            Li = LMAX - 8 * i
            flat = o[i].reshape(NKC_MAX * 128, NRIC)[:Li]  # (l - 8i, f)
            out[0, :, 8 * i :, m] = (flat[:, :C] + 1j * flat[:, C:]).T
    return out


# revision 15
# speedup vs baseline: 1.4725x; 1.0134x over previous
"""Distributed real SHT (spherical harmonic transform) on 8 trn2 NeuronCores.

Pipeline:
  out[b,c,l,m] = sum_k W[m,l,k] * XF[b,c,m,k],   XF = (2*pi/nlon) * rfft(x, lon)[..., :mmax]

Stage A (launch 1, channel-sharded): DFT along longitude as bf16 matmuls.
  Host folds x over lon parity (cos: n'=0..360, sin: n'=1..359) and packs
  GROUPS of 4 channels per DMA so every transfer is >=0.6 MB with >=2.9 KB
  contiguous per-partition runs (descriptor-efficient; single-queue BW was
  measured 112 GB/s at 0.7 KB runs vs 200 GB/s at 4.3 KB).
  psum[k_tile, m] += xT[n'chunk, k_tile]^T @ DFTmat[n'chunk, m]
Host exchange: XF[c,k,m] (channel-sharded) -> per-core m-sharded, p-major
  chunk layout (partition-major so per-partition runs are nkc*1KB).
Stage B (launch 2, m-sharded): psum[l_tile, 512] += WT[k, l_tile]^T @ XFB[k, 512]
  Only the latitude window where P_l^m is non-negligible is loaded and
  contracted, with EXACT spans (partial last chunk via a separate DMA +
  partial-partition matmul) instead of 128-row padding.

All DMA traffic is spread across the three queues (sync/scalar HWDGE +
gpsimd SWDGE) roughly by their measured throughputs. bf16 operands keep
the PE at 2.4 GHz and halve DMA bytes; psum accumulation is fp32.
"""

import os

import numpy as np

import concourse.bacc as bacc
import concourse.mybir as mybir
from concourse.tile import TileContext
from concourse.bass_utils import run_bass_kernel_spmd

LAST_PERF = {}

NLAT = 361
NLON = 720
MMAX = 361
LMAX = 361
C = 256
NCORES = 8
CPC = C // NCORES  # 32 channels per core
NC_COS = NLON // 2 + 1  # 361 cos columns (n'=0..360)
NC_SIN = NLON // 2 - 1  # 359 sin columns (n'=1..359)
MPC = (MMAX + NCORES - 1) // NCORES  # 46 m's per core (padded)
MEVEN = 362  # m padded even (stage A moving free dim)
GA = 4  # channels per stage-A DMA group
NGA = CPC // GA  # 8 groups per core
NKC_MAX = 3  # max 128-row latitude chunks in stage B
NRIC = 2 * C  # 512 = (re|im) x 256 channels

F32 = mybir.dt.float32
BF16 = mybir.dt.bfloat16

K_TILES = [(0, 128), (128, 128), (256, 105)]


def _ptiles(n, p=128):
    out = []
    o = 0
    while o < n:
        out.append((o, min(p, n - o)))
        o += p
    return out


def build_stage_a():
    """xin [NGA, 768, GA*362] bf16: row r = DFT contraction row (cos rows
    0:361 in segs 0-2, sin rows 384:743 in segs 3-5), col = c*362 + k.
    mats [768, 362] bf16 (same row packing, cols = m, col 361 zero).
    xf [NGA, 361, GA*724]: row k, col = c*724 + ri*362 + m."""
    nc = bacc.Bacc("TRN2", target_bir_lowering=False)
    xin = nc.dram_tensor("xin", [NGA, 768, GA * MEVEN], BF16, kind="ExternalInput")
    mats = nc.dram_tensor("mats", [768, MEVEN], BF16, kind="ExternalInput")
    xf = nc.dram_tensor("xf", [NGA, NLAT, GA * 2 * MEVEN], BF16, kind="ExternalOutput")

    with TileContext(nc) as tc:
        with (
            tc.tile_pool(name="mats", bufs=1) as matp,
            tc.tile_pool(name="xinp", bufs=4) as xinp,
            tc.tile_pool(name="outp", bufs=6) as outp,
            tc.tile_pool(name="ps", bufs=6, space="PSUM") as psp,
        ):
            mat_t = matp.tile([128, 6 * MEVEN], BF16, tag="mats")
            # on scalar so it runs concurrently with group 0's load on sync
            nc.scalar.dma_start(
                out=mat_t.rearrange("p (s m) -> p s m", s=6),
                in_=mats.rearrange("(s p) m -> p s m", p=128),
            )
            copy_i = 0
            for g in range(NGA):
                x_t = xinp.tile([128, 6 * GA * MEVEN], BF16, tag="xin")
                (nc.sync if g % 2 == 0 else nc.scalar).dma_start(
                    out=x_t.rearrange("p (s f) -> p s f", s=6),
                    in_=xin[g].rearrange("(s p) f -> p s f", p=128),
                )
                ots = [
                    outp.tile([128, GA * 2 * MEVEN], BF16, tag="ot", name=f"ot{kt}")
                    for kt in range(len(K_TILES))
                ]
                for c in range(GA):
                    for ri in range(2):
                        for kt, (k0, kp) in enumerate(K_TILES):
                            ps = psp.tile([128, MEVEN], F32, tag="ps")
                            for s in range(3):
                                seg = 3 * ri + s
                                base = (seg * GA + c) * MEVEN
                                nc.tensor.matmul(
                                    ps[:kp, :],
                                    x_t[:, base + k0 : base + k0 + kp],
                                    mat_t[:, seg * MEVEN : (seg + 1) * MEVEN],
                                    start=(s == 0),
                                    stop=(s == 2),
                                )
                            dst = ots[kt][:kp, (c * 2 + ri) * MEVEN : (c * 2 + ri + 1) * MEVEN]
                            if copy_i % 3 != 2:  # 2/3 DVE, 1/3 ACT (ACT is ~2x slower)
                                nc.vector.tensor_copy(out=dst, in_=ps[:kp, :])
                            else:
                                nc.scalar.copy(dst, ps[:kp, :])
                            copy_i += 1
                for kt, (k0, kp) in enumerate(K_TILES):
                    # kt0/kt1 -> gpsimd; kt2 -> the HWDGE engine that is NOT
                    # about to issue the next group's load
                    if kt < 2:
                        st = nc.gpsimd
                    else:
                        st = nc.scalar if g % 2 == 0 else nc.sync
                    st.dma_start(out=xf[g, k0 : k0 + kp, :], in_=ots[kt][:kp, :])
    nc.compile()
    return nc


def build_stage_b(nkc_list, r_list):
    """Index i handles m = 8*i + core_j; computes l in [8*i, lmax).
    xfb [MPC, 128, 3*512] bf16, p-major: col t*512+f holds XF latitude row
    (klo_i + t*128 + p), f = ri*256 + c.  wt [MPC, 128, 1088]: col t*Lp+lc
    holds W[m, same k row, l_lo_i + lc] (Lp = lmax - 8*(i & ~1); odd i's
    last 8 cols zero).  outb [MPC, 128, 3*512]: col t*512+f holds out row
    l = 8*i + t*128 + p.
    nkc_list[i]: latitude chunks (uniform within each index pair);
    r_list[i]: valid rows of the last chunk (span - 128*(nkc-1))."""
    nc = bacc.Bacc("TRN2", target_bir_lowering=False)
    xfb = nc.dram_tensor("xfb", [MPC, 128, NKC_MAX * NRIC], BF16, kind="ExternalInput")
    wt = nc.dram_tensor("wt", [MPC, 128, 1088], BF16, kind="ExternalInput")
    outb = nc.dram_tensor("outb", [MPC, 128, NKC_MAX * NRIC], BF16, kind="ExternalOutput")

    with TileContext(nc) as tc:
        with (
            tc.tile_pool(name="rhs", bufs=6) as rhsp,
            tc.tile_pool(name="wts", bufs=6) as wtp,
            tc.tile_pool(name="outp", bufs=6) as outp,
            tc.tile_pool(name="ps", bufs=7, space="PSUM") as psp,
        ):
            copy_i = 0
            for pi in range(0, MPC, 2):
                nkc = nkc_list[pi]
                Lp = LMAX - 8 * pi  # col width shared within the pair
                x_t = rhsp.tile([128, 2 * NKC_MAX * NRIC], BF16, tag="rhs")
                w_t = wtp.tile([128, 2 * 1088], BF16, tag="wt")
                ot = outp.tile([128, 2 * NKC_MAX * NRIC], BF16, tag="ot")
                eng_a = nc.sync if (pi // 2) % 2 == 0 else nc.scalar
                eng_b = nc.scalar if (pi // 2) % 2 == 0 else nc.sync
                # pair-batched whole-index loads (~0.5-0.8 MB per transfer,
                # 2-3 KB contiguous per partition run); rows beyond each
                # window are host-zero-filled in BOTH tensors, so full-128
                # contraction is exact.
                eng_a.dma_start(
                    out=x_t.rearrange("p (i f) -> p i f", i=2)[:, :, : nkc * NRIC],
                    in_=xfb[pi : pi + 2, :, : nkc * NRIC].rearrange("i p f -> p i f"),
                )
                eng_b.dma_start(
                    out=w_t.rearrange("p (i f) -> p i f", i=2)[:, :, : nkc * Lp],
                    in_=wt[pi : pi + 2, :, : nkc * Lp].rearrange("i p f -> p i f"),
                )
                for il in range(2):
                    Li = LMAX - 8 * (pi + il)
                    for tl, (l0, lp) in enumerate(_ptiles(Li)):
                        ps = psp.tile([128, NRIC], F32, tag="ps")
                        for t in range(nkc):
                            nc.tensor.matmul(
                                ps[:lp, :],
                                w_t[:, il * 1088 + t * Lp + l0 : il * 1088 + t * Lp + l0 + lp],
                                x_t[:, il * NKC_MAX * NRIC + t * NRIC : il * NKC_MAX * NRIC + (t + 1) * NRIC],
                                start=(t == 0),
                                stop=(t == nkc - 1),
                            )
                        dst = ot[:lp, (il * NKC_MAX + tl) * NRIC : (il * NKC_MAX + tl + 1) * NRIC]
                        if copy_i % 3 != 2:  # 2/3 DVE, 1/3 ACT (ACT is ~2x slower)
                            nc.vector.tensor_copy(out=dst, in_=ps[:lp, :])
                        else:
                            nc.scalar.copy(dst, ps[:lp, :])
                        copy_i += 1
                # stores on gpsimd only: never block the HWDGE load queues.
                # full l-tiles batched across the pair; partial tiles per-index
                nfull = len(_ptiles(LMAX - 8 * pi)) - 1  # same within the pair
                if nfull > 0:
                    nc.gpsimd.dma_start(
                        out=outb[pi : pi + 2, :, : nfull * NRIC].rearrange(
                            "i p f -> p i f"
                        ),
                        in_=ot.rearrange("p (i f) -> p i f", i=2)[:, :, : nfull * NRIC],
                    )
                for il in range(2):
                    lp_last = _ptiles(LMAX - 8 * (pi + il))[-1][1]
                    nc.gpsimd.dma_start(
                        out=outb[pi + il, :lp_last, nfull * NRIC : (nfull + 1) * NRIC],
                        in_=ot[
                            :lp_last,
                            (il * NKC_MAX + nfull) * NRIC : (il * NKC_MAX + nfull + 1) * NRIC,
                        ],
                    )
    nc.compile()
    return nc


def _dft_matrices():
    """cosm[n', m] = s*cos(2 pi m n'/nlon), n'=0..360
    sinm[n', m] = -s*sin(2 pi m n'/nlon), n'=1..359 (imag of rfft = -sum x sin)."""
    s = 2.0 * np.pi / NLON
    m = np.arange(MMAX)
    nc_ = np.arange(NC_COS)
    ns_ = np.arange(1, NLON // 2)
    ang_c = 2.0 * np.pi * ((nc_[:, None] * m[None, :]) % NLON) / NLON
    ang_s = 2.0 * np.pi * ((ns_[:, None] * m[None, :]) % NLON) / NLON
    return (s * np.cos(ang_c)).astype(np.float32), (-s * np.sin(ang_s)).astype(
        np.float32
    )


def fold_x(x):
    """x: (C, nlat, nlon) f32 -> xc (C, nlat, 361), xs (C, nlat, 359)."""
    xc = np.empty((x.shape[0], x.shape[1], NC_COS), dtype=np.float32)
    xc[..., 0] = x[..., 0]
    xc[..., NLON // 2] = x[..., NLON // 2]
    xc[..., 1 : NLON // 2] = x[..., 1 : NLON // 2] + x[..., : NLON // 2 : -1]
    xs = x[..., 1 : NLON // 2] - x[..., : NLON // 2 : -1]
    return xc, np.ascontiguousarray(xs.astype(np.float32))


def pack_stage_a_inputs(x):
    """x: (C, nlat, nlon) f32 -> xin (C//GA, 768, GA*362) bf16, mats (768, 362)."""
    import ml_dtypes

    bf = ml_dtypes.bfloat16
    xc, xs = fold_x(x)  # (C, k, n')
    ng = x.shape[0] // GA
    xin = np.zeros((ng, 768, GA, MEVEN), dtype=bf)
    # [g, n', c, k] <- transpose of (g, c, k, n')
    xin[:, :NC_COS, :, :NLAT] = (
        xc.reshape(ng, GA, NLAT, NC_COS).transpose(0, 3, 1, 2).astype(bf)
    )
    xin[:, 384 : 384 + NC_SIN, :, :NLAT] = (
        xs.reshape(ng, GA, NLAT, NC_SIN).transpose(0, 3, 1, 2).astype(bf)
    )
    cosm, sinm = _dft_matrices()
    mats = np.zeros((768, MEVEN), dtype=bf)
    mats[:NC_COS, :MMAX] = cosm.astype(bf)
    mats[384 : 384 + NC_SIN, :MMAX] = sinm.astype(bf)
    return xin.reshape(ng, 768, GA * MEVEN), mats


def _windows(weights):
    """Per index-pair latitude windows: union of |W| support over the 8
    cores' m's, span forced uniform (in chunk count) within each pair."""
    wabs = np.abs(weights).max(axis=1)  # (m, k)
    thr = 1e-7 * wabs.max()
    win = []
    for i in range(MPC):
        ms = [NCORES * i + j for j in range(NCORES) if NCORES * i + j < MMAX]
        nz = np.nonzero(wabs[ms].max(axis=0) > thr)[0]
        klo, khi = (int(nz[0]), int(nz[-1]) + 1) if len(nz) else (0, NLAT)
        win.append([klo, khi])
    nkc_list, r_list, klo_list = [], [], []
    for pi in range(0, MPC, 2):
        nkc = max(-(-(w[1] - w[0]) // 128) for w in win[pi : pi + 2])
        for i in (pi, pi + 1):
            klo, khi = win[i]
            need = (nkc - 1) * 128 + 16  # keep the partial chunk >= 16 rows
            if khi - klo < need:
                khi = min(NLAT, klo + need)
                klo = max(0, khi - need)
            nkc_list.append(nkc)
            r_list.append(khi - klo - 128 * (nkc - 1))
            klo_list.append(klo)
    return nkc_list, r_list, klo_list


def m_list(j):
    return [NCORES * i + j for i in range(MPC) if NCORES * i + j < MMAX]


def _install_ntff_hook():
    """This image's antenv lacks axon_hooks; synthesize it so bass_utils'
    trace=True path can capture NTFFs via the axon PJRT .so."""
    import sys

    if "antenv.axon_hooks" in sys.modules:
        return
    import types

    mod = types.ModuleType("antenv.axon_hooks")
    state = {"hook": None}
    mod.set_axon_ntff_profile_hook = lambda h: state.__setitem__("hook", h)
    mod.get_axon_ntff_profile_hook = lambda: state["hook"]
    sys.modules["antenv.axon_hooks"] = mod
    try:
        import importlib.util as ilu

        spec = ilu.spec_from_file_location(
            "_trn_boot_hook", "/root/.axon_site/trn_agent_boot/trn_boot.py"
        )
        tb = ilu.module_from_spec(spec)
        spec.loader.exec_module(tb)
        mod.set_axon_ntff_profile_hook(
            tb._ntff_profile_via_ctypes("/opt/axon/libaxon_pjrt.so")
        )
    except Exception:
        pass


def _run(nc, in_maps, label):
    kw = {}
    if os.environ.get("SHT_TRACE"):
        import concourse.bass_utils as bu

        bu.upload_artifacts = lambda tmpdir: tmpdir  # no S3 in this sandbox
        _install_ntff_hook()
        kw = dict(trace=True)
    try:
        res = run_bass_kernel_spmd(nc, in_maps, core_ids=list(range(NCORES)), **kw)
    except Exception:
        if not kw:
            raise
        res = run_bass_kernel_spmd(nc, in_maps, core_ids=list(range(NCORES)))
    LAST_PERF[label] = res.exec_time_ns
    return res


def kernel(x, weights):
    import ml_dtypes

    bf = ml_dtypes.bfloat16
    x = np.asarray(x, dtype=np.float32).reshape(C, NLAT, NLON)
    weights = np.asarray(weights, dtype=np.float32)

    xin, mats = pack_stage_a_inputs(x)
    nc_a = build_stage_a()
    in_maps = [
        {"xin": xin[j * NGA : (j + 1) * NGA], "mats": mats} for j in range(NCORES)
    ]
    res_a = _run(nc_a, in_maps, "stage_a")
    # reassemble XF[c, k, m] re/im from [g, k, c*724 + ri*362 + m]
    xf_all = np.concatenate(
        [np.asarray(r["xf"]).reshape(NGA, NLAT, GA, 2, MEVEN) for r in res_a.results],
        axis=0,
    )  # (C//GA, k, GA, ri, m)
    xf_all = xf_all.transpose(0, 2, 3, 1, 4).reshape(C, 2, NLAT, MEVEN)
    xfr = np.ascontiguousarray(xf_all[:, 0, :, :MMAX])  # (C, k, m) bf16
    xfi = np.ascontiguousarray(xf_all[:, 1, :, :MMAX])

    nkc_list, r_list, klo_list = _windows(weights)
    wtf = weights.transpose(0, 2, 1)  # (m, k, l) f32
    in_maps_b = []
    for j in range(NCORES):
        xfb = np.zeros((MPC, 128, NKC_MAX * NRIC), dtype=bf)
        wtj = np.zeros((MPC, 128, 1088), dtype=bf)
        for i in range(MPC):
            m = NCORES * i + j
            if m >= MMAX:
                continue
            nkc, klo = nkc_list[i], klo_list[i]
            span = 128 * (nkc - 1) + r_list[i]
            khi = klo + span
            Lp = LMAX - 8 * (i & ~1)
            l_lo = 8 * i
            # (span, 512) -> chunk-padded (nkc*128, 512) -> p-major (128, nkc, 512)
            src = np.empty((span, NRIC), dtype=bf)
            src[:, :C] = xfr[:, klo:khi, m].T
            src[:, C:] = xfi[:, klo:khi, m].T
            pad = np.zeros((nkc * 128, NRIC), dtype=bf)
            pad[:span] = src
            xfb[i, :, : nkc * NRIC] = pad.reshape(nkc, 128, NRIC).transpose(1, 0, 2).reshape(128, nkc * NRIC)
            wsrc = np.zeros((nkc * 128, Lp), dtype=bf)
            wsrc[:span, : LMAX - l_lo] = wtf[m, klo:khi, l_lo:].astype(bf)
            wtj[i, :, : nkc * Lp] = wsrc.reshape(nkc, 128, Lp).transpose(1, 0, 2).reshape(128, nkc * Lp)
        in_maps_b.append({"xfb": xfb, "wt": wtj})
    nc_b = build_stage_b(nkc_list, r_list)
    res_b = _run(nc_b, in_maps_b, "stage_b")

    out = np.zeros((1, C, LMAX, MMAX), dtype=np.complex64)
    for j in range(NCORES):
        o = np.asarray(res_b.results[j]["outb"], dtype=np.float32)  # (MPC,128,1536)
        o = o.reshape(MPC, 128, NKC_MAX, NRIC).transpose(0, 2, 1, 3)  # (i, tl, p, f)
        for i in range(MPC):
            m = NCORES * i + j
            if m >= MMAX:
                continue
            Li = LMAX - 8 * i
            flat = o[i].reshape(NKC_MAX * 128, NRIC)[:Li]  # (l - 8i, f)
            out[0, :, 8 * i :, m] = (flat[:, :C] + 1j * flat[:, C:]).T
    return out
